# revision 1
# baseline (speedup 1.0000x reference)
"""Trainium2 Bass kernel for nn_Conv2d_77489799955262.

Forward value of the reference:
    y = conv2d(x, (w_pos > 0) - (w_neg > 0))      # ternary weights in {-1, 0, 1}
(the straight-through-estimator terms cancel numerically), NCHW, 3x3, stride 1,
pad 1, x [32, 256, 56, 56] f32, w [256, 256, 3, 3].

Strategy: data-parallel over batch across 8 cores (4 images per core).
Implicit GEMM: for each of the 9 taps and each 128-wide ci block, one
[ci=128, co=128] stationary matmul accumulates into PSUM over a moving window
of the zero-padded input image [58 x 58]. Output rows are processed in blocks
of 8 -> each matmul has N = 8*58 = 464 (2 garbage columns per row are computed
and later dropped by a strided PSUM drain).

dtype modes:
  "f32r"  - single pass with float32r matmuls (full PE rate at N >= 256).
  "bf16x2"- x split into bf16 hi + lo parts (products with ternary weights are
            exact in bf16, so only the lo-rounding ~2^-16 relative remains);
            two accumulation passes per tap/ci at full bf16 rate.
"""
import numpy as np
import ml_dtypes

import concourse.bass as bass
import concourse.tile as tile
from concourse import bacc, mybir
from concourse.bass_utils import run_bass_kernel_spmd

MODE = "f32r"  # "f32r" | "bf16x2"

N_CORES = 8
B, CI, CO, H, W, K = 32, 256, 256, 56, 56, 3
NI = B // N_CORES          # images per core
PH, PW = H + 2, W + 1      # padded rows; row STRIDE is 57: the right pad of
                           # row r and the left pad of row r+1 share one zero
IMG = PH * PW              # 3306 padded elems per image per channel
CIB = CI // 128            # ci blocks
COB = CO // 128            # co blocks
RB = 8                     # output rows per psum tile
NMM = RB * PW              # matmul moving free dim: 456
NRB = H // RB              # 7 row blocks per image
SLACK = 64                 # trailing zeros so edge windows stay in-bounds
XLEN = CIB * NI * IMG + SLACK

F32 = mybir.dt.float32
F32R = mybir.dt.float32r
BF16 = mybir.dt.bfloat16

_COMPILED = {}


def _build(mode, iters=1, loop=0):
    # "f32r": x and w both float32r.  "mixed": x float32r, w bfloat16
    # (ternary weights are exact in bf16, and 2-byte weights get FWL).
    # "bf16x2": x split hi/lo bf16, w bf16, two passes.
    is_split = mode == "bf16x2"
    xdt = BF16 if is_split else F32R
    wdt = BF16 if mode in ("bf16x2", "mixed") else F32R
    nparts = 2 if is_split else 1

    nc = bacc.Bacc("TRN2", target_bir_lowering=False, debug=False,
                   num_devices=N_CORES)

    x_in_dt = F32 if is_split else F32R
    x_dram = nc.dram_tensor("x", [NI, CI, H, W], x_in_dt, kind="ExternalInput")
    w_dram = nc.dram_tensor("w", [CI, 9, CO], wdt, kind="ExternalInput")
    y_dram = nc.dram_tensor("y", [NI, CO, H, W], F32, kind="ExternalOutput")

    with tile.TileContext(nc) as tc:
        with (
            tc.tile_pool(name="const", bufs=1) as cpool,
            tc.tile_pool(name="stage", bufs=3) as spool,
            tc.tile_pool(name="outp", bufs=3) as opool,
            tc.tile_pool(name="psum", bufs=8, space="PSUM") as ppool,
        ):
            # first-needed weight slice ahead of x on the sync queue; the
            # rest on the gpsimd (SWDGE) queue, ordered by first use
            w_sb = cpool.tile([128, CIB, 9, CO], wdt, tag="w")

            def w_slice(ci, co):
                return (w_sb[:, ci, :, co * 128:(co + 1) * 128],
                        w_dram[ci * 128:(ci + 1) * 128, :,
                               co * 128:(co + 1) * 128])

            nc.sync.dma_start(*w_slice(0, 0))
            for ci, co in ((1, 0), (0, 1), (1, 1)):
                nc.gpsimd.dma_start(*w_slice(ci, co))

            # padded input planes, one flat free dim per part
            parts = [cpool.tile([128, XLEN], xdt, tag=f"xp{p}", name=f"xp{p}")
                     for p in range(nparts)]

            def xoff(ci, n):
                return ci * NI * IMG + n * IMG

            # memset via a plain-f32/bf16 view: walrus rejects f32r memsets
            mdt = F32 if xdt == F32R else xdt

            def emit_iter(it):
                # zero only the pad borders (top row + (1,0); the (r,57),(r+1,0)
                # pair column; (56,57) + bottom row) + trailing slack, on the
                # otherwise-idle gpsimd engine so DVE stays free for drains
                for p in range(nparts):
                    for ci in range(CIB):
                        for n in range(NI):
                            # image 0 on DVE (idle at start; drains come much
                            # later) so the pool queue's w DMAs don't gate it
                            eng = nc.vector if n == 0 else nc.gpsimd
                            o = xoff(ci, n)
                            # top row + (1,0); the single pad col of rows
                            # 2..56; bottom row
                            eng.memset(
                                parts[p][:, o:o + PW + 1].bitcast(mdt), 0.0)
                            cols = (parts[p][:, o + 2 * PW:
                                             o + 2 * PW + 55 * PW]
                                    .rearrange("p (r c) -> p r c", c=PW)
                                    [:, :, 0:1])
                            eng.memset(cols.bitcast(mdt), 0.0)
                            eng.memset(
                                parts[p][:, o + 57 * PW:o + IMG]
                                .bitcast(mdt), 0.0)
                    if it == 0:
                        nc.gpsimd.memset(
                            parts[p][:, CIB * NI * IMG:].bitcast(mdt), 0.0)

                for n in range(NI):
                    for ci in range(CIB):
                        interior = (
                            parts[0][:, xoff(ci, n):xoff(ci, n) + IMG]
                            .rearrange("p (r c) -> p r c", c=PW)
                            [:, 1:1 + H, 1:1 + W]
                        )
                        src = x_dram[n, ci * 128:(ci + 1) * 128, :, :]
                        if not is_split:
                            if n == 0:
                                # split image 0 so the first matmul group's
                                # rows (0..32 + halo) land sooner
                                nc.sync.dma_start(interior[:, 0:33, :],
                                                  src[:, 0:33, :])
                                nc.sync.dma_start(interior[:, 33:H, :],
                                                  src[:, 33:H, :])
                            else:
                                nc.sync.dma_start(interior, src)
                        else:
                            stg = spool.tile([128, H * W], F32, tag="stg",
                                             name=f"stg_{it}_{n}_{ci}")
                            nc.sync.dma_start(stg[:], src)
                            stg_v = stg[:].rearrange("p (r c) -> p r c", c=W)
                            lo_interior = (
                                parts[1][:, xoff(ci, n):xoff(ci, n) + IMG]
                                .rearrange("p (r c) -> p r c", c=PW)
                                [:, 1:1 + H, 1:1 + W]
                            )
                            nc.vector.tensor_copy(interior, stg_v)  # f32->bf16
                            nc.vector.tensor_sub(lo_interior, stg_v, interior)

                # main matmul loops
                for n in range(NI):
                    for co in range(COB):
                        for g0, gn in ((0, 4), (4, 3)):
                            pss = [ppool.tile([128, NMM], F32, tag="ps",
                                              name=f"ps_{it}_{n}_{co}_{g0}_{r}")
                                   for r in range(gn)]
                            n_seg = 9 * CIB * nparts
                            seg = 0
                            for ci in range(CIB):
                                for tap in range(9):
                                    kh, kw = tap // 3, tap % 3
                                    lhsT = w_sb[:, ci, tap,
                                                co * 128:(co + 1) * 128]
                                    for p in range(nparts):
                                        for r in range(gn):
                                            r0 = (g0 + r) * RB
                                            base = (xoff(ci, n)
                                                    + (r0 + kh) * PW + kw)
                                            nc.tensor.matmul(
                                                pss[r][:], lhsT,
                                                parts[p][:, base:base + NMM],
                                                start=(seg == 0),
                                                stop=(seg == n_seg - 1))
                                        seg += 1
                            ot = opool.tile([128, gn * RB * W], F32, tag="ot",
                                            name=f"ot_{it}_{n}_{co}_{g0}")
                            ot_v = ot[:].rearrange("p (r c) -> p r c", c=W)
                            for r in range(gn):
                                src = (pss[r][:]
                                       .rearrange("p (i j) -> p i j", j=PW)
                                       [:, :, 0:W])
                                nc.vector.tensor_copy(
                                    ot_v[:, r * RB:(r + 1) * RB, :], src)
                            nc.sync.dma_start(
                                y_dram[n, co * 128:(co + 1) * 128,
                                       g0 * RB:(g0 + gn) * RB, :],
                                ot[:])

            if loop:
                with tc.For_i(0, loop, 1,
                              hint_engines=(mybir.EngineType.PE,)):
                    emit_iter(0)
            else:
                for it in range(iters):
                    emit_iter(it)

    nc.compile()
    return nc


def _get_compiled(mode):
    if mode not in _COMPILED:
        _COMPILED[mode] = _build(mode)
    return _COMPILED[mode]


def _prep_weights(w_pos, w_neg, mode):
    w_eff = ((w_pos > 0).astype(np.float32)
             - (w_neg > 0).astype(np.float32))          # [CO, CI, 3, 3]
    w_lhsT = np.ascontiguousarray(
        w_eff.reshape(CO, CI, 9).transpose(1, 2, 0))    # [CI, 9, CO]
    if mode == "f32r":
        return w_lhsT
    return w_lhsT.astype(ml_dtypes.bfloat16)           # exact for {-1,0,1}


def kernel(x, w_pos, w_neg):
    mode = MODE
    nc = _get_compiled(mode)
    w_lhsT = _prep_weights(w_pos, w_neg, mode)
    x = np.ascontiguousarray(x, dtype=np.float32)

    in_maps = [
        {"x": x[c * NI:(c + 1) * NI], "w": w_lhsT}
        for c in range(N_CORES)
    ]
    res = run_bass_kernel_spmd(nc, in_maps, list(range(N_CORES)))
    out = np.concatenate([res.results[c]["y"] for c in range(N_CORES)], axis=0)
    return out.astype(np.float32)

